# revision 1
# baseline (speedup 1.0000x reference)
"""Trainium2 Bass kernel for nn_MultiHeadAttention_66322884984909.

Math (faithful to reference):
  Q = X @ W_Q.T reshaped (B, H, L, hd) via DIRECT reshape -> head h owns rows
  128h:128(h+1) of the projected (L, D) matrix, reinterpreted as (L=2048, hd=64).
  Heads are therefore sequence-parallel: the whole computation decomposes over
  the 32 (batch, head) pairs with no cross-pair coupling. 8 cores x 4 pairs.

Per pair (X_s = X[b, 128h:128h+128, :], shape (128, 1024)):
  Qf = X_s @ W_Q.T        (128, 1024)  -> Qh = Qf.reshape(2048, 64)
  S  = Qh @ Kh.T          (2048, 2048) causal-masked softmax (no scaling)
  O  = softmax(S) @ Vh    (2048, 64)
  Y  = O.reshape(128, 1024) @ W_O.T + b_O   -> out rows 128h:128(h+1) of batch b

No max-subtraction in softmax: logits ~ N(0, 64), |S| < 80 with overwhelming
probability, exp stays finite in fp32. Row sums come free as a 65th ones-column
appended to V in the P@V matmul. All matmuls run in fp32r (full-rate tensor
engine mode, ~1e-4 relative error).
"""

import numpy as np

import concourse.bass as bass
from concourse import bacc
import concourse.mybir as mybir
import concourse.tile as tile
from concourse.bass_utils import run_bass_kernel_spmd
from concourse.masks import make_identity

F32 = mybir.dt.float32
F32R = mybir.dt.float32r
EXP = mybir.ActivationFunctionType.Exp

B, L, D = 2, 2048, 1024
H, HD = 16, 64
NCORES = 8
PPC = 4  # pairs per core


def build_nc(repeat=1):
    nc = bacc.Bacc(trn_type="TRN2", target_bir_lowering=False, debug=False)

    xt = nc.declare_dram_parameter("xt", [PPC, 1024, 128], F32R, isOutput=False)
    wq = nc.declare_dram_parameter("wq", [1024, 1024], F32R, isOutput=False)   # W_Q.T
    wk = nc.declare_dram_parameter("wk", [1024, 1024], F32R, isOutput=False)   # W_K.T
    wv = nc.declare_dram_parameter("wv", [1024, 1024], F32R, isOutput=False)   # W_V.T
    wo = nc.declare_dram_parameter("wo", [64, 16 * 1024], F32R, isOutput=False)
    bias = nc.declare_dram_parameter("bias", [128, 1024], F32, isOutput=False)
    ones = nc.declare_dram_parameter("ones", [128, 16], F32R, isOutput=False)
    out = nc.declare_dram_parameter("out", [PPC, 128, 1024], F32, isOutput=True)
    qsh = nc.dram_tensor("qsh", [PPC // 2, 128, 2048], F32R)
    ksh = nc.dram_tensor("ksh", [PPC // 2, 128, 2048], F32R)
    vsh = nc.dram_tensor("vsh", [PPC, 128, 1040], F32R)

    with tile.TileContext(nc) as tc:
      for _rep in range(repeat):
        with (
            tc.tile_pool(name="consts", bufs=1) as consts,
            tc.tile_pool(name="headt", bufs=1) as headt,
            tc.tile_pool(name="mmps", bufs=4, space="PSUM") as mmps,
            tc.tile_pool(name="stps", bufs=1, space="PSUM") as stps,
            tc.tile_pool(name="onp", bufs=2) as onp,
            tc.tile_pool(name="ptp", bufs=4) as ptp,
            tc.tile_pool(name="rp", bufs=4) as rp,
            tc.tile_pool(name="yp", bufs=2) as ypool,
        ):
            bias_sb = consts.tile([128, 1024], F32)
            nc.sync.dma_start(out=bias_sb, in_=bias[:])
            ident_f = consts.tile([128, 128], F32)
            make_identity(nc, ident_f)
            ident = consts.tile([128, 128], F32R)
            nc.vector.tensor_copy(ident, ident_f)

            NG = PPC // 2
            qht2 = [headt.tile([128, 2048], F32R, tag=f"qht{g}", name=f"qht{g}")
                    for g in range(NG)]
            kht2 = [headt.tile([128, 2048], F32R, tag=f"kht{g}", name=f"kht{g}")
                    for g in range(NG)]
            vh = [headt.tile([128, 16 * 65], F32R, tag=f"vh{p}", name=f"vh{p}")
                  for p in range(PPC)]

            def emit_phases(xt_sb, pwork):
                """Projections + shuffles + transposes for all pairs."""
                for (phase_i, wparam, sh, is_v) in (
                        (0, wq, qsh, False), (1, wk, ksh, False),
                        (2, wv, vsh, True)):
                    with tc.tile_pool(name=f"pw{phase_i}", bufs=1) as pw:
                        w_sb = pw.tile([128, 8, 1024], F32R, tag="w",
                                       name=f"w{phase_i}")
                        for kc in range(8):
                            nc.sync.dma_start(
                                out=w_sb[:, kc, :],
                                in_=wparam.rearrange(
                                    "(c p) j -> p c j", p=128)[:, kc, :])
                      # loop groups inside the weight phase
                        for g2 in range(PPC // 2):
                          if is_v:
                              for ii in range(2):
                                  p = 2 * g2 + ii
                                  nat = pwork.tile([128, 1024], F32R, tag="natv",
                                                   bufs=2, name=f"natv{p}")
                                  for jh in range(2):
                                      ps = mmps.tile([128, 512], F32, tag="mm",
                                                     name="projps")
                                      for kc in range(8):
                                          nc.tensor.matmul(
                                              ps,
                                              lhsT=xt_sb[2 * g2 + ii][:, kc, :],
                                              rhs=w_sb[:, kc,
                                                       jh * 512:(jh + 1) * 512],
                                              start=(kc == 0), stop=(kc == 7),
                                          )
                                      nc.vector.tensor_copy(
                                          nat[:, jh * 512:(jh + 1) * 512], ps)
                                  shr = sh[p].rearrange(
                                      "(il pp2) (t j) -> t il pp2 j",
                                      il=8, t=16)[:, :, :, 0:64]
                                  nc.gpsimd.dma_start(out=shr, in_=nat[:])
                                  nc.sync.dma_start(out=vh[p][:], in_=sh[p])
                                  nc.gpsimd.dma_start(
                                      out=vh[p].rearrange(
                                          "q (b c) -> q b c", c=65)[:, :, 64],
                                      in_=ones[:])  # ones column at 65b+64
                          else:
                              # pair-interleaved (pp, pair, j2) scratch: bounce
                              # write streams 512B-contiguous runs
                              nat2 = pwork.tile([128, 16, 2, 64], F32R, tag="nat",
                                                bufs=3, name=f"nat2_{g2}_{phase_i}")
                              for ii in range(2):
                                  p = 2 * g2 + ii
                                  for jh in range(2):
                                      ps = mmps.tile([128, 512], F32, tag="mm",
                                                     name="projps")
                                      for kc in range(8):
                                          nc.tensor.matmul(
                                              ps,
                                              lhsT=xt_sb[2 * g2 + ii][:, kc, :],
                                              rhs=w_sb[:, kc,
                                                       jh * 512:(jh + 1) * 512],
                                              start=(kc == 0), stop=(kc == 7),
                                          )
                                      nc.vector.tensor_copy(
                                          nat2[:, 8 * jh:8 * (jh + 1), ii, :], ps)
                              shr = sh[g2].rearrange(
                                  "(il pp2) (t w j) -> t il pp2 w j",
                                  il=8, t=16, w=2)
                              nc.gpsimd.dma_start(out=shr, in_=nat2[:])
                              hh2 = pwork.tile([128, 2048], F32R, tag="hh",
                                               bufs=2, name=f"hh{g2}_{phase_i}")
                              nc.sync.dma_start(out=hh2[:], in_=sh[g2])
                              dst = qht2[g2] if phase_i == 0 else kht2[g2]
                              for bt in range(4):
                                  tb = stps.tile([128, 512], F32R, tag="stA",
                                                 name="trps")
                                  for j in range(4):
                                      ti = 4 * bt + j
                                      nc.tensor.transpose(
                                          tb[:, j * 128:(j + 1) * 128],
                                          hh2[:, ti * 128:(ti + 1) * 128],
                                          ident,
                                      )
                                  nc.scalar.copy(
                                      dst[:, bt * 512:(bt + 1) * 512], tb)

            def emit_attention(g):
                onorm2 = onp.tile([128, 2048], F32R, tag="onorm",
                                  name=f"onorm{g}")
                for a in range(4):
                    pvs = [mmps.tile([65, 512], F32, tag="mm",
                                     name=f"pv_{i}") for i in range(2)]
                    for gg in range(2 * a + 2):
                        sts = [stps.tile([128, 1024], F32, tag=t_,
                                         name=f"st{t_}")
                               for t_ in ("stA", "stB")]
                        for q2 in range(2):
                            bb = 2 * gg + q2
                            for i in range(2):
                                nc.tensor.matmul(
                                    sts[i][:, q2 * 512:(q2 + 1) * 512],
                                    lhsT=kht2[g][64 * i:64 * i + 64,
                                                 bb * 128:(bb + 1) * 128],
                                    rhs=qht2[g][64 * i:64 * i + 64,
                                                a * 512:(a + 1) * 512],
                                    start=True, stop=True,
                                )
                        for i in range(2):
                            pt = ptp.tile([128, 1024], F32R, tag="pt",
                                          name=f"pt_{i}")
                            nc.scalar.activation(pt, sts[i], EXP)
                            if gg >= 2 * a:  # diagonal: causal mask
                                r0 = 2 * (gg - 2 * a)
                                nc.gpsimd.affine_select(
                                    out=pt.rearrange("q (w j) -> q w j", w=2),
                                    in_=pt.rearrange("q (w j) -> q w j", w=2),
                                    compare_op=mybir.AluOpType.is_ge,
                                    fill=0.0,
                                    base=-128 * r0,
                                    pattern=[[-128, 2], [1, 512]],
                                    channel_multiplier=-1,
                                )
                            for q2 in range(2):
                                bb = 2 * gg + q2
                                nc.tensor.matmul(
                                    pvs[i],
                                    lhsT=vh[2 * g + i][:, bb * 65:bb * 65 + 65],
                                    rhs=pt[:, q2 * 512:(q2 + 1) * 512],
                                    start=(bb == 0), stop=(bb == 4 * a + 3),
                                )
                    for i in range(2):
                        r1 = rp.tile([1, 512], F32, tag="r1", name="r1_t")
                        nc.vector.reciprocal(r1, pvs[i][64:65, :])
                        rb = rp.tile([64, 512], F32, tag="rb", name="rb_t")
                        nc.gpsimd.partition_broadcast(rb, r1)
                        nc.vector.tensor_mul(
                            onorm2[64 * i:64 * i + 64, a * 512:(a + 1) * 512],
                            pvs[i][0:64, :], rb)

                return onorm2

            def emit_y(g, onorm2, wo_sb):
                # row-packed output projection for both pairs of the group
                onorm_r = onorm2.rearrange("q (i t) -> q t i", t=16)
                ysbs = [ypool.tile([128, 1024], F32, tag="ysb",
                                   name=f"ysb{g}_{i}") for i in range(2)]
                for jh in range(2):
                    yps = [mmps.tile([128, 512], F32, tag="mm",
                                     name=f"ypsum_{i}") for i in range(2)]
                    for t in range(16):
                        for i in range(2):
                            nc.tensor.matmul(
                                yps[i],
                                lhsT=onorm_r[64 * i:64 * i + 64, t, :],
                                rhs=wo_sb[64 * i:64 * i + 64,
                                          t * 1024 + jh * 512:
                                          t * 1024 + (jh + 1) * 512],
                                start=(t == 0), stop=(t == 15),
                            )
                    for i in range(2):
                        nc.vector.tensor_add(
                            ysbs[i][:, jh * 512:(jh + 1) * 512], yps[i],
                            bias_sb[:, jh * 512:(jh + 1) * 512])
                for i in range(2):
                    nc.sync.dma_start(out=out[2 * g + i], in_=ysbs[i])

            # pipeline: group-0 phases; group-1 phases overlap group-0
            # attention (DMA is idle during attention)
            with tile.TileContext.tile_pool(tc, name="xtp", bufs=1) as xtp, \
                 tile.TileContext.tile_pool(tc, name="pwork", bufs=1) as pwork:
                xt_sb = []
                for p in range(PPC):
                    t = xtp.tile([128, 8, 128], F32R, tag=f"xt{p}", name=f"xtsb{p}")
                    nc.scalar.dma_start(
                        out=t, in_=xt[p].rearrange("(c p) i -> p c i", p=128))
                    xt_sb.append(t)
                emit_phases(xt_sb, pwork)

            with tc.tile_pool(name="p2", bufs=1) as p2:
                wo_sb = p2.tile([128, 16 * 1024], F32R, tag="wo")
                for wc in range(8):
                    nc.sync.dma_start(out=wo_sb[0:64, wc * 2048:(wc + 1) * 2048],
                                      in_=wo[:, wc * 2048:(wc + 1) * 2048])
                    nc.vector.tensor_copy(
                        wo_sb[64:128, wc * 2048:(wc + 1) * 2048],
                        wo_sb[0:64, wc * 2048:(wc + 1) * 2048])
                for g in range(NG):
                    onorm2 = emit_attention(g)
                    emit_y(g, onorm2, wo_sb)

    nc.finalize()




    return nc


def _host_prep(input_seq_embs, W_Q, W_K, W_V, W_O, b_O):
    X = np.asarray(input_seq_embs, dtype=np.float32)
    WQ = np.asarray(W_Q, dtype=np.float32)
    WK = np.asarray(W_K, dtype=np.float32)
    WV = np.asarray(W_V, dtype=np.float32)
    WO = np.asarray(W_O, dtype=np.float32)
    bO = np.asarray(b_O, dtype=np.float32)

    wq_arr = np.ascontiguousarray(WQ.T)
    wk_arr = np.ascontiguousarray(WK.T)
    wv_arr = np.ascontiguousarray(WV.T)
    # wo[j2, 1024 t + jo] = W_O.T[64 t + j2, jo]
    wo_arr = np.ascontiguousarray(
        WO.T.reshape(16, 64, 1024).transpose(1, 0, 2).reshape(64, 16 * 1024))
    bias_arr = np.ascontiguousarray(
        np.broadcast_to(bO, (128, 1024)).astype(np.float32))

    in_maps = []
    for c in range(NCORES):
        xts = []
        for p in range(PPC):
            g = PPC * c + p
            bb, hh = g // H, g % H
            xts.append(np.ascontiguousarray(X[bb, 128 * hh:128 * (hh + 1), :].T))
        in_maps.append({
            "xt": np.stack(xts),
            "wq": wq_arr, "wk": wk_arr, "wv": wv_arr, "wo": wo_arr,
            "bias": bias_arr,
            "ones": np.ones((128, 16), dtype=np.float32),
        })
    return in_maps


_CACHED_NC = None


def get_nc():
    global _CACHED_NC
    if _CACHED_NC is None:
        _CACHED_NC = build_nc()
    return _CACHED_NC


def kernel(**inputs) -> np.ndarray:
    nc = get_nc()
    in_maps = _host_prep(**inputs)
    res = run_bass_kernel_spmd(nc, in_maps, list(range(NCORES)))
    out = np.empty((B, L, D), dtype=np.float32)
    for c in range(NCORES):
        y = res.results[c]["out"]  # (4, 128, 1024)
        for p in range(PPC):
            g = PPC * c + p
            bb, hh = g // H, g % H
            out[bb, 128 * hh:128 * (hh + 1), :] = y[p]
    return out

